# revision 4
# baseline (speedup 1.0000x reference)
"""Trainium2 Bass kernel for Bottleneck+DynamicConv (B=16,C=256,H=W=64,E=4).

Data-parallel over batch: 8 NeuronCores x 2 samples each. Each 3x3 conv is
128x128x512 matmuls: weights stationary per (tap, cin-tile, cout-tile)
block, the moving operand is a 3D access pattern [128, 8 rows, 64 cols]
into a zero-padded 66x66 image held in SBUF, so taps are pure AP offsets
and only valid output pixels are computed. BN scales are folded into conv
weights on the host; the BN bias + SiLU epilogue runs fused on the scalar
engine during PSUM evacuation. Compute dtype fp16 (same PE rate as bf16,
8x finer rounding), accumulation fp32 in PSUM.
"""

from contextlib import ExitStack

import numpy as np

import concourse.bacc as bacc
import concourse.bass as bass
import concourse.mybir as mybir
from concourse import tile
from concourse.bass_utils import run_bass_kernel_spmd

B, C, H, W, E = 16, 256, 64, 64, 4
KH = KW = 3
EPS = 1e-5
NCORES = 8
S = B // NCORES           # samples per core = 2
CT = C // 128             # channel tiles = 2
PD = W + 2                # padded width/height = 66
PF = PD * PD              # padded flat pixels per channel tile = 4356
NGB = 8                   # row-blocks per (sample, cout tile)
RB = H // NGB             # rows per block = 8
NN = RB * W               # matmul free dim = 512
HWF = H * W               # 4096
NBLK = CT * 9 * CT        # 36 weight blocks of [128,128]
BLKF = NBLK * 128         # 4608 weight columns
F16 = mybir.dt.float16
F32 = mybir.dt.float32
NPF16 = np.float16

TRACE = False
LAST_EXEC_NS = None
# swappable for simulator testing (CoreSim has no Silu); HW uses native Silu
ACT_FUNC = mybir.ActivationFunctionType.Silu

_prog_cache = {}


def _blk(o, t, ci):
    # column block index in the packed weight layout [128, 36*128]
    return (o * 9 + t) * 2 + ci


def _emit_conv(nc, ps_pool, wcol, views, epilogue):
    """One conv layer for one sample. wcol(o,t,ci) -> lhsT AP;
    views[ci] = [128, 66, 66] padded-input AP; epilogue(o, gb, psum_ap)."""
    for o in range(CT):
        for gb in range(NGB):
            r0 = gb * RB
            ps = ps_pool.tile([128, NN], F32, tag="ps")
            idx = 0
            for ci in range(CT):
                for t in range(9):
                    dy, dx = t // 3 - 1, t % 3 - 1
                    rhs = views[ci][:, r0 + 1 + dy:r0 + 1 + dy + RB,
                                    1 + dx:1 + dx + W]
                    nc.tensor.matmul(
                        ps[:], wcol(o, t, ci), rhs,
                        start=(idx == 0), stop=(idx == 17))
                    idx += 1
            epilogue(o, gb, ps)


def _build_program():
    nc = bacc.Bacc(
        "TRN2", target_bir_lowering=False, debug=False,
        enable_asserts=False, num_devices=NCORES)

    xpad_d = nc.dram_tensor("xpad", [S, CT, 128, PF], F16, kind="ExternalInput")
    w1_d = nc.dram_tensor("w1sb", [128, BLKF], F16, kind="ExternalInput")
    bank_d = nc.dram_tensor("bank", [128, E * BLKF], F16, kind="ExternalInput")
    wr_d = nc.dram_tensor("wrt", [128, CT * E], F32, kind="ExternalInput")
    br_d = nc.dram_tensor("brb", [128, E], F32, kind="ExternalInput")
    b1_d = nc.dram_tensor("b1sb", [128, CT], F32, kind="ExternalInput")
    b2_d = nc.dram_tensor("b2sb", [128, CT], F32, kind="ExternalInput")
    out_d = nc.dram_tensor("out", [S, CT, 128, HWF], F32, kind="ExternalOutput")

    with tile.TileContext(nc) as tc, ExitStack() as ctx:
        const = ctx.enter_context(tc.tile_pool(name="const", bufs=1))
        xp_pool = ctx.enter_context(tc.tile_pool(name="xp", bufs=2))
        yp_pool = ctx.enter_context(tc.tile_pool(name="yp", bufs=2))
        kern_pool = ctx.enter_context(tc.tile_pool(name="kern", bufs=2))
        outp_pool = ctx.enter_context(tc.tile_pool(name="outp", bufs=2))
        small = ctx.enter_context(tc.tile_pool(name="small", bufs=4))
        ps_pool = ctx.enter_context(tc.tile_pool(name="ps", bufs=4, space="PSUM"))
        psr_pool = ctx.enter_context(tc.tile_pool(name="psr", bufs=2, space="PSUM"))

        # startup-critical DMA order: the very first matmuls need only the
        # o=0 weight blocks and the leading image rows of sample 0 — load
        # those first so the PE starts ~10us earlier. The first 6 blocks go
        # in their own transfer so LDWEIGHTS can begin before the rest land.
        w1_t = const.tile([128, BLKF], F16)
        W1C0 = 6 * 128
        nc.scalar.dma_start(w1_t[:, 0:W1C0], w1_d.ap()[:, 0:W1C0])
        b1_t = const.tile([128, CT], F32)
        nc.sync.dma_start(b1_t[:], b1_d.ap())

        # dummy matmuls on a memset tile: keep the PE busy from ~7us so the
        # HAM clock-gate flips to 2.4 GHz before the real matmuls start
        warm_t = const.tile([128, 128], F16)
        nc.vector.memset(warm_t[:], 0.0)
        for wi in range(48):
            wps = ps_pool.tile([128, 64], F32, tag="ps")
            nc.tensor.matmul(wps[:], warm_t[:], warm_t[:, 0:64],
                             start=True, stop=True)
        wr_t = const.tile([128, CT * E], F32)
        br_t = const.tile([128, E], F32)
        b2_t = const.tile([128, CT], F32)
        ones_t = const.tile([128, 128], F32)
        nc.vector.memset(ones_t[:], 1.0)
        # the big expert bank is only needed after conv1(s0): allocate now,
        # DMA later so it doesn't delay the startup-critical loads above
        bank_t = const.tile([128, E * BLKF], F16)

        # s0 image quarters land in consumption order (row-blocks 0..1 need
        # rows <18, blocks 2..3 rows <34, ...)
        QS = [0, 18 * PD, 34 * PD, 50 * PD, PF]
        xpts, ypts, kerns = [], [], []
        for s in range(S):
            xpt = xp_pool.tile([128, CT * PF], F16, tag="xp")
            xpts.append(xpt)
            pieces = (list(zip(QS[:-1], QS[1:])) if s == 0 else [(0, PF)])
            for qi, (lo, hi) in enumerate(pieces):
                # split image quarters across both HWDGE rings (SP + ACT)
                for ci in range(CT):
                    eng = nc.sync if ci == 0 else nc.scalar
                    eng.dma_start(
                        xpt[:, ci * PF + lo:ci * PF + hi],
                        xpad_d.ap()[s, ci][:, lo:hi])
                if s == 0 and qi == 0:
                    nc.scalar.dma_start(
                        w1_t[:, W1C0:BLKF // 2], w1_d.ap()[:, W1C0:BLKF // 2])
                if s == 0 and qi == 1:
                    nc.scalar.dma_start(
                        w1_t[:, BLKF // 2:], w1_d.ap()[:, BLKF // 2:])
            if s == 0:
                nc.sync.dma_start(wr_t[:], wr_d.ap())
                nc.sync.dma_start(br_t[:], br_d.ap())
                nc.sync.dma_start(b2_t[:], b2_d.ap())
            xviews = [xpt[:, ci * PF:(ci + 1) * PF].rearrange(
                "p (h w) -> p h w", h=PD) for ci in range(CT)]

            ypt = yp_pool.tile([128, CT * PF], F16, tag="yp")
            ypts.append(ypt)
            yviews = [ypt[:, ci * PF:(ci + 1) * PF].rearrange(
                "p (h w) -> p h w", h=PD) for ci in range(CT)]

            def epi1(o, gb, ps, yviews=yviews):
                r0 = gb * RB
                nc.scalar.activation(
                    yviews[o][:, r0 + 1:r0 + 1 + RB, 1:1 + W],
                    ps[:].rearrange("p (a b) -> p a b", a=RB),
                    ACT_FUNC, bias=b1_t[:, o:o + 1])

            _emit_conv(
                nc, ps_pool,
                lambda o, t, ci: w1_t[:, _blk(o, t, ci) * 128:(_blk(o, t, ci) + 1) * 128],
                xviews, epi1)

            if s == 0:
                nc.sync.dma_start(bank_t[:], bank_d.ap())

            # zero the pad ring of y so conv2 sees proper zero padding
            for ci in range(CT):
                yv = yviews[ci]
                nc.vector.memset(yv[:, 0:1, :], 0.0)
                nc.vector.memset(yv[:, PD - 1:PD, :], 0.0)
                nc.vector.memset(yv[:, :, 0:1], 0.0)
                nc.vector.memset(yv[:, :, PD - 1:PD], 0.0)

            # routing: pooled mean -> sigmoid(pooled @ wrT + br), all 128
            # partitions carry identical copies (broadcast via ones-matmul)
            psr = psr_pool.tile([128, E], F32, tag="psr")
            for ci in range(CT):
                pooled = small.tile([128, 1], F32, tag="pooled")
                nc.vector.tensor_reduce(
                    pooled[:], yviews[ci][:, 1:1 + H, 1:1 + W],
                    axis=mybir.AxisListType.XY, op=mybir.AluOpType.add)
                pbc = small.tile([128, 128], F32, tag="pbc")
                nc.vector.tensor_scalar_mul(pbc[:], ones_t[:], pooled[:, 0:1])
                nc.tensor.matmul(
                    psr[:], pbc[:], wr_t[:, ci * E:(ci + 1) * E],
                    start=(ci == 0), stop=(ci == CT - 1))
            logits = small.tile([128, E], F32, tag="logits")
            nc.vector.tensor_add(logits[:], psr[:], br_t[:])
            r_t = small.tile([128, E], F32, tag="r")
            nc.scalar.activation(
                r_t[:], logits[:], mybir.ActivationFunctionType.Sigmoid)

            # expert-weighted kernel bank mix: kern = sum_e r_e * bank_e
            kt = kern_pool.tile([128, BLKF], F16, tag="kern")
            kerns.append(kt)
            nc.vector.tensor_scalar_mul(kt[:], bank_t[:, 0:BLKF], r_t[:, 0:1])
            for e in range(1, E):
                nc.vector.scalar_tensor_tensor(
                    kt[:], bank_t[:, e * BLKF:(e + 1) * BLKF], r_t[:, e:e + 1],
                    kt[:], mybir.AluOpType.mult, mybir.AluOpType.add)

        for s in range(S):
            ypt, kt = ypts[s], kerns[s]
            yviews = [ypt[:, ci * PF:(ci + 1) * PF].rearrange(
                "p (h w) -> p h w", h=PD) for ci in range(CT)]
            outps = {}
            xpt = xpts[s]
            xviews2 = [xpt[:, ci * PF:(ci + 1) * PF].rearrange(
                "p (h w) -> p h w", h=PD) for ci in range(CT)]

            # epilogue does silu+bias, residual add and writeback per 512-px
            # chunk so the output DMA overlaps the remaining matmuls
            def epi2(o, gb, ps):
                if gb == 0:
                    outps[o] = outp_pool.tile(
                        [128, HWF], F32, tag="outp", name=f"outp_s{s}_o{o}")
                sl = bass.ts(gb, NN)
                nc.scalar.activation(
                    outps[o][:, sl], ps[:],
                    ACT_FUNC, bias=b2_t[:, o:o + 1])
                nc.vector.tensor_add(
                    outps[o][:, sl].rearrange("p (a b) -> p a b", a=RB),
                    outps[o][:, sl].rearrange("p (a b) -> p a b", a=RB),
                    xviews2[o][:, 1 + gb * RB:1 + (gb + 1) * RB, 1:1 + W])
                nc.sync.dma_start(
                    out_d.ap()[s, o][:, sl], outps[o][:, sl])

            _emit_conv(
                nc, ps_pool,
                lambda o, t, ci: kt[:, _blk(o, t, ci) * 128:(_blk(o, t, ci) + 1) * 128],
                yviews, epi2)

    nc.compile()
    return nc


def _get_program():
    if "nc" not in _prog_cache:
        _prog_cache["nc"] = _build_program()
    return _prog_cache["nc"]


def kernel(x, w1, bn1_g, bn1_b, bn1_m, bn1_v, wr, br, w_e,
           bn2_g, bn2_b, bn2_m, bn2_v):
    global LAST_EXEC_NS
    f32 = np.float32
    x = np.ascontiguousarray(np.asarray(x, f32))
    w1 = np.asarray(w1, f32)
    wr = np.asarray(wr, f32)
    br = np.asarray(br, f32)
    w_e = np.asarray(w_e, f32)

    s1 = np.asarray(bn1_g, f32) / np.sqrt(np.asarray(bn1_v, f32) + EPS)
    b1 = np.asarray(bn1_b, f32) - np.asarray(bn1_m, f32) * s1
    s2 = np.asarray(bn2_g, f32) / np.sqrt(np.asarray(bn2_v, f32) + EPS)
    b2 = np.asarray(bn2_b, f32) - np.asarray(bn2_m, f32) * s2

    # pack conv1 weights [cout, cin, ky, kx] (BN1 scale folded) into the
    # lhsT block layout: [cin128 partitions, (o, ky, kx, ci, cout128)]
    w1f = w1 * s1[:, None, None, None]
    w1sb = np.ascontiguousarray(
        w1f.reshape(CT, 128, CT, 128, KH, KW)
        .transpose(3, 0, 4, 5, 2, 1).reshape(128, BLKF)).astype(NPF16)

    # expert bank likewise (BN2 scale folded), one block set per expert
    wef = w_e.reshape(E, C, C, KH, KW) * s2[None, :, None, None, None]
    bank = np.ascontiguousarray(
        wef.reshape(E, CT, 128, CT, 128, KH, KW)
        .transpose(4, 0, 1, 5, 6, 3, 2).reshape(128, E * BLKF)).astype(NPF16)

    # routing weights with the 1/(H*W) mean folded in: [p, (ci, e)]
    wrt = np.ascontiguousarray(
        (wr / HWF).reshape(E, CT, 128).transpose(2, 1, 0).reshape(128, CT * E))
    brb = np.ascontiguousarray(np.broadcast_to(br, (128, E)))
    b1sb = np.ascontiguousarray(b1.reshape(CT, 128).T)
    b2sb = np.ascontiguousarray(b2.reshape(CT, 128).T)

    # padded fp16 x for the conv matmuls (also reused as the residual)
    pad = np.zeros((B, CT, 128, PD, PD), f32)
    pad[:, :, :, 1:H + 1, 1:W + 1] = x.reshape(B, CT, 128, H, W)
    xpad = np.ascontiguousarray(pad.reshape(B, CT, 128, PF).astype(NPF16))

    nc = _get_program()
    in_maps = []
    for c in range(NCORES):
        sl = slice(S * c, S * (c + 1))
        in_maps.append({
            "xpad": np.ascontiguousarray(xpad[sl]),
            "w1sb": w1sb, "bank": bank, "wrt": wrt, "brb": brb,
            "b1sb": b1sb, "b2sb": b2sb,
        })

    res = run_bass_kernel_spmd(
        nc, in_maps, core_ids=list(range(NCORES)), trace=TRACE)
    LAST_EXEC_NS = res.exec_time_ns

    out = np.empty((B, C, H, W), f32)
    for c in range(NCORES):
        out[S * c:S * (c + 1)] = res.results[c]["out"].reshape(S, C, H, W)
    return out


if __name__ == "__main__":
    rng = np.random.default_rng(0)
    ins = {
        "x": rng.standard_normal((B, C, H, W), f32 := np.float32),
        "w1": rng.standard_normal((C, C, KH, KW), f32) * 0.05,
        "bn1_g": np.ones(C, f32), "bn1_b": np.zeros(C, f32),
        "bn1_m": rng.standard_normal(C, f32) * 0.05,
        "bn1_v": np.abs(rng.standard_normal(C, f32) * 0.05) + 1.0,
        "wr": rng.standard_normal((E, C), f32) * 0.05,
        "br": np.zeros(E, f32),
        "w_e": rng.standard_normal((E, C * C * KH * KW), f32) * 0.05,
        "bn2_g": np.ones(C, f32), "bn2_b": np.zeros(C, f32),
        "bn2_m": rng.standard_normal(C, f32) * 0.05,
        "bn2_v": np.abs(rng.standard_normal(C, f32) * 0.05) + 1.0,
    }
    o = kernel(**ins)
    print(o.shape, o.dtype)



# revision 6
# speedup vs baseline: 1.2638x; 1.2638x over previous
"""Trainium2 Bass kernel for Bottleneck+DynamicConv (B=16,C=256,H=W=64,E=4).

Data-parallel over batch: 8 NeuronCores x 2 samples each. Each 3x3 conv is
128x128x512 matmuls: weights stationary per (tap, cin-tile, cout-tile)
block, the moving operand is a 3D access pattern [128, 8 rows, 64 cols]
into a zero-padded 66x66 image held in SBUF, so taps are pure AP offsets
and only valid output pixels are computed. BN scales are folded into conv
weights on the host; the BN bias + SiLU epilogue runs fused on the scalar
engine during PSUM evacuation. Compute dtype fp16 (same PE rate as bf16,
8x finer rounding), accumulation fp32 in PSUM.
"""

from contextlib import ExitStack

import numpy as np

import concourse.bacc as bacc
import concourse.bass as bass
import concourse.mybir as mybir
from concourse import tile
from concourse.bass_utils import run_bass_kernel_spmd

B, C, H, W, E = 16, 256, 64, 64, 4
KH = KW = 3
EPS = 1e-5
NCORES = 8
S = B // NCORES           # samples per core = 2
CT = C // 128             # channel tiles = 2
PD = W + 2                # padded width/height = 66
PF = PD * PD              # padded flat pixels per channel tile = 4356
NGB = 8                   # row-blocks per (sample, cout tile)
RB = H // NGB             # rows per block = 8
NN = RB * W               # matmul free dim = 512
HWF = H * W               # 4096
NBLK = CT * 9 * CT        # 36 weight blocks of [128,128]
BLKF = NBLK * 128         # 4608 weight columns
F16 = mybir.dt.float16
F32 = mybir.dt.float32
NPF16 = np.float16

TRACE = False
LAST_EXEC_NS = None
# swappable for simulator testing (CoreSim has no Silu); HW uses native Silu
ACT_FUNC = mybir.ActivationFunctionType.Silu

_prog_cache = {}


def _blk(o, t, ci):
    # column block index in the packed weight layout [128, 36*128]
    return (o * 9 + t) * 2 + ci


def _emit_conv(nc, ps_pool, wcol, views, epilogue):
    """One conv layer for one sample. wcol(o,t,ci) -> lhsT AP;
    views[ci] = [128, 66, 66] padded-input AP; epilogue(o, gb, psum_ap)."""
    for o in range(CT):
        for gb in range(NGB):
            r0 = gb * RB
            ps = ps_pool.tile([128, NN], F32, tag="ps")
            idx = 0
            for ci in range(CT):
                for t in range(9):
                    dy, dx = t // 3 - 1, t % 3 - 1
                    rhs = views[ci][:, r0 + 1 + dy:r0 + 1 + dy + RB,
                                    1 + dx:1 + dx + W]
                    nc.tensor.matmul(
                        ps[:], wcol(o, t, ci), rhs,
                        start=(idx == 0), stop=(idx == 17))
                    idx += 1
            epilogue(o, gb, ps)


def _build_program():
    nc = bacc.Bacc(
        "TRN2", target_bir_lowering=False, debug=False,
        enable_asserts=False, num_devices=NCORES)

    xpad_d = nc.dram_tensor("xpad", [S, CT, 128, PF], F16, kind="ExternalInput")
    w1_d = nc.dram_tensor("w1sb", [128, BLKF], F16, kind="ExternalInput")
    bank_d = nc.dram_tensor("bank", [128, E * BLKF], F16, kind="ExternalInput")
    wr_d = nc.dram_tensor("wrt", [128, CT * E], F32, kind="ExternalInput")
    br_d = nc.dram_tensor("brb", [128, E], F32, kind="ExternalInput")
    b1_d = nc.dram_tensor("b1sb", [128, CT], F32, kind="ExternalInput")
    b2_d = nc.dram_tensor("b2sb", [128, CT], F32, kind="ExternalInput")
    out_d = nc.dram_tensor("out", [S, CT, 128, HWF], F32, kind="ExternalOutput")

    with tile.TileContext(nc) as tc, ExitStack() as ctx:
        const = ctx.enter_context(tc.tile_pool(name="const", bufs=1))
        xp_pool = ctx.enter_context(tc.tile_pool(name="xp", bufs=2))
        yp_pool = ctx.enter_context(tc.tile_pool(name="yp", bufs=2))
        kern_pool = ctx.enter_context(tc.tile_pool(name="kern", bufs=2))
        outp_pool = ctx.enter_context(tc.tile_pool(name="outp", bufs=2))
        small = ctx.enter_context(tc.tile_pool(name="small", bufs=4))
        ps_pool = ctx.enter_context(tc.tile_pool(name="ps", bufs=4, space="PSUM"))
        psr_pool = ctx.enter_context(tc.tile_pool(name="psr", bufs=2, space="PSUM"))

        # startup-critical DMA order: the very first matmuls need only the
        # o=0 weight blocks and the leading image rows of sample 0 — load
        # those first so the PE starts ~10us earlier. The first 6 blocks go
        # in their own transfer so LDWEIGHTS can begin before the rest land.
        w1_t = const.tile([128, BLKF], F16)
        nc.scalar.dma_start(w1_t[:, 0:BLKF // 2], w1_d.ap()[:, 0:BLKF // 2])
        b1_t = const.tile([128, CT], F32)
        nc.sync.dma_start(b1_t[:], b1_d.ap())

        # dense dummy matmuls while the startup DMAs stream: keeps the PE
        # busy from ~7us so the HAM clock-gate is at 2.4 GHz when real
        # matmuls start (cold ramp otherwise costs ~2.5us)
        warm_t = const.tile([128, 512], F16)
        nc.vector.memset(warm_t[:], 0.0)
        for wi in range(15):
            wps = ps_pool.tile([128, NN], F32, tag="ps")
            nc.tensor.matmul(wps[:], warm_t[:, 0:128], warm_t[:],
                             start=True, stop=True)
        wr_t = const.tile([128, CT * E], F32)
        br_t = const.tile([128, E], F32)
        b2_t = const.tile([128, CT], F32)
        ones_t = const.tile([128, 128], F32)
        nc.vector.memset(ones_t[:], 1.0)
        # the big expert bank is only needed after conv1(s0): allocate now,
        # DMA later so it doesn't delay the startup-critical loads above
        bank_t = const.tile([128, E * BLKF], F16)

        # s0 image quarters land in consumption order (row-blocks 0..1 need
        # rows <18, blocks 2..3 rows <34, ...)
        QS = [0, 18 * PD, 34 * PD, 50 * PD, PF]
        xpts, ypts, kerns = [], [], []
        for s in range(S):
            xpt = xp_pool.tile([128, CT * PF], F16, tag="xp")
            xpts.append(xpt)
            pieces = (list(zip(QS[:-1], QS[1:])) if s == 0 else [(0, PF)])
            for qi, (lo, hi) in enumerate(pieces):
                # split image quarters across both HWDGE rings (SP + ACT);
                # the startup-critical s0 quarter 0 goes fully on sync so it
                # doesn't queue behind the 590KB w1 half on scalar
                for ci in range(CT):
                    eng = nc.scalar if (ci == 1 and not (s == 0 and qi == 0)) \
                        else nc.sync
                    eng.dma_start(
                        xpt[:, ci * PF + lo:ci * PF + hi],
                        xpad_d.ap()[s, ci][:, lo:hi])
                if s == 0 and qi == 1:
                    nc.scalar.dma_start(
                        w1_t[:, BLKF // 2:], w1_d.ap()[:, BLKF // 2:])
            if s == 0:
                nc.sync.dma_start(wr_t[:], wr_d.ap())
                nc.sync.dma_start(br_t[:], br_d.ap())
                nc.sync.dma_start(b2_t[:], b2_d.ap())
            xviews = [xpt[:, ci * PF:(ci + 1) * PF].rearrange(
                "p (h w) -> p h w", h=PD) for ci in range(CT)]

            ypt = yp_pool.tile([128, CT * PF], F16, tag="yp")
            ypts.append(ypt)
            yviews = [ypt[:, ci * PF:(ci + 1) * PF].rearrange(
                "p (h w) -> p h w", h=PD) for ci in range(CT)]

            def epi1(o, gb, ps, yviews=yviews):
                r0 = gb * RB
                nc.scalar.activation(
                    yviews[o][:, r0 + 1:r0 + 1 + RB, 1:1 + W],
                    ps[:].rearrange("p (a b) -> p a b", a=RB),
                    ACT_FUNC, bias=b1_t[:, o:o + 1])

            _emit_conv(
                nc, ps_pool,
                lambda o, t, ci: w1_t[:, _blk(o, t, ci) * 128:(_blk(o, t, ci) + 1) * 128],
                xviews, epi1)

            if s == 0:
                nc.sync.dma_start(bank_t[:], bank_d.ap())

            # zero the pad ring of y so conv2 sees proper zero padding
            for ci in range(CT):
                yv = yviews[ci]
                nc.vector.memset(yv[:, 0:1, :], 0.0)
                nc.vector.memset(yv[:, PD - 1:PD, :], 0.0)
                nc.vector.memset(yv[:, :, 0:1], 0.0)
                nc.vector.memset(yv[:, :, PD - 1:PD], 0.0)

            # routing: pooled mean -> sigmoid(pooled @ wrT + br), all 128
            # partitions carry identical copies (broadcast via ones-matmul)
            psr = psr_pool.tile([128, E], F32, tag="psr")
            for ci in range(CT):
                pooled = small.tile([128, 1], F32, tag="pooled")
                nc.vector.tensor_reduce(
                    pooled[:], yviews[ci][:, 1:1 + H, 1:1 + W],
                    axis=mybir.AxisListType.XY, op=mybir.AluOpType.add)
                pbc = small.tile([128, 128], F32, tag="pbc")
                nc.vector.tensor_scalar_mul(pbc[:], ones_t[:], pooled[:, 0:1])
                nc.tensor.matmul(
                    psr[:], pbc[:], wr_t[:, ci * E:(ci + 1) * E],
                    start=(ci == 0), stop=(ci == CT - 1))
            logits = small.tile([128, E], F32, tag="logits")
            nc.vector.tensor_add(logits[:], psr[:], br_t[:])
            r_t = small.tile([128, E], F32, tag="r")
            nc.scalar.activation(
                r_t[:], logits[:], mybir.ActivationFunctionType.Sigmoid)

            # expert-weighted kernel bank mix: kern = sum_e r_e * bank_e
            kt = kern_pool.tile([128, BLKF], F16, tag="kern")
            kerns.append(kt)
            nc.vector.tensor_scalar_mul(kt[:], bank_t[:, 0:BLKF], r_t[:, 0:1])
            for e in range(1, E):
                nc.vector.scalar_tensor_tensor(
                    kt[:], bank_t[:, e * BLKF:(e + 1) * BLKF], r_t[:, e:e + 1],
                    kt[:], mybir.AluOpType.mult, mybir.AluOpType.add)

        for s in range(S):
            ypt, kt = ypts[s], kerns[s]
            yviews = [ypt[:, ci * PF:(ci + 1) * PF].rearrange(
                "p (h w) -> p h w", h=PD) for ci in range(CT)]
            outps = {}
            xpt = xpts[s]
            xviews2 = [xpt[:, ci * PF:(ci + 1) * PF].rearrange(
                "p (h w) -> p h w", h=PD) for ci in range(CT)]

            # epilogue does silu+bias, residual add and writeback per 512-px
            # chunk so the output DMA overlaps the remaining matmuls
            def epi2(o, gb, ps):
                if gb == 0:
                    outps[o] = outp_pool.tile(
                        [128, HWF], F32, tag="outp", name=f"outp_s{s}_o{o}")
                sl = bass.ts(gb, NN)
                nc.scalar.activation(
                    outps[o][:, sl], ps[:],
                    ACT_FUNC, bias=b2_t[:, o:o + 1])
                nc.vector.tensor_add(
                    outps[o][:, sl].rearrange("p (a b) -> p a b", a=RB),
                    outps[o][:, sl].rearrange("p (a b) -> p a b", a=RB),
                    xviews2[o][:, 1 + gb * RB:1 + (gb + 1) * RB, 1:1 + W])
                nc.sync.dma_start(
                    out_d.ap()[s, o][:, sl], outps[o][:, sl])

            _emit_conv(
                nc, ps_pool,
                lambda o, t, ci: kt[:, _blk(o, t, ci) * 128:(_blk(o, t, ci) + 1) * 128],
                yviews, epi2)

    nc.compile()
    return nc


def _get_program():
    if "nc" not in _prog_cache:
        _prog_cache["nc"] = _build_program()
    return _prog_cache["nc"]


def kernel(x, w1, bn1_g, bn1_b, bn1_m, bn1_v, wr, br, w_e,
           bn2_g, bn2_b, bn2_m, bn2_v):
    global LAST_EXEC_NS
    f32 = np.float32
    x = np.ascontiguousarray(np.asarray(x, f32))
    w1 = np.asarray(w1, f32)
    wr = np.asarray(wr, f32)
    br = np.asarray(br, f32)
    w_e = np.asarray(w_e, f32)

    s1 = np.asarray(bn1_g, f32) / np.sqrt(np.asarray(bn1_v, f32) + EPS)
    b1 = np.asarray(bn1_b, f32) - np.asarray(bn1_m, f32) * s1
    s2 = np.asarray(bn2_g, f32) / np.sqrt(np.asarray(bn2_v, f32) + EPS)
    b2 = np.asarray(bn2_b, f32) - np.asarray(bn2_m, f32) * s2

    # pack conv1 weights [cout, cin, ky, kx] (BN1 scale folded) into the
    # lhsT block layout: [cin128 partitions, (o, ky, kx, ci, cout128)]
    w1f = w1 * s1[:, None, None, None]
    w1sb = np.ascontiguousarray(
        w1f.reshape(CT, 128, CT, 128, KH, KW)
        .transpose(3, 0, 4, 5, 2, 1).reshape(128, BLKF)).astype(NPF16)

    # expert bank likewise (BN2 scale folded), one block set per expert
    wef = w_e.reshape(E, C, C, KH, KW) * s2[None, :, None, None, None]
    bank = np.ascontiguousarray(
        wef.reshape(E, CT, 128, CT, 128, KH, KW)
        .transpose(4, 0, 1, 5, 6, 3, 2).reshape(128, E * BLKF)).astype(NPF16)

    # routing weights with the 1/(H*W) mean folded in: [p, (ci, e)]
    wrt = np.ascontiguousarray(
        (wr / HWF).reshape(E, CT, 128).transpose(2, 1, 0).reshape(128, CT * E))
    brb = np.ascontiguousarray(np.broadcast_to(br, (128, E)))
    b1sb = np.ascontiguousarray(b1.reshape(CT, 128).T)
    b2sb = np.ascontiguousarray(b2.reshape(CT, 128).T)

    # padded fp16 x for the conv matmuls (also reused as the residual)
    pad = np.zeros((B, CT, 128, PD, PD), f32)
    pad[:, :, :, 1:H + 1, 1:W + 1] = x.reshape(B, CT, 128, H, W)
    xpad = np.ascontiguousarray(pad.reshape(B, CT, 128, PF).astype(NPF16))

    nc = _get_program()
    in_maps = []
    for c in range(NCORES):
        sl = slice(S * c, S * (c + 1))
        in_maps.append({
            "xpad": np.ascontiguousarray(xpad[sl]),
            "w1sb": w1sb, "bank": bank, "wrt": wrt, "brb": brb,
            "b1sb": b1sb, "b2sb": b2sb,
        })

    res = run_bass_kernel_spmd(
        nc, in_maps, core_ids=list(range(NCORES)), trace=TRACE)
    LAST_EXEC_NS = res.exec_time_ns

    out = np.empty((B, C, H, W), f32)
    for c in range(NCORES):
        out[S * c:S * (c + 1)] = res.results[c]["out"].reshape(S, C, H, W)
    return out


if __name__ == "__main__":
    rng = np.random.default_rng(0)
    ins = {
        "x": rng.standard_normal((B, C, H, W), f32 := np.float32),
        "w1": rng.standard_normal((C, C, KH, KW), f32) * 0.05,
        "bn1_g": np.ones(C, f32), "bn1_b": np.zeros(C, f32),
        "bn1_m": rng.standard_normal(C, f32) * 0.05,
        "bn1_v": np.abs(rng.standard_normal(C, f32) * 0.05) + 1.0,
        "wr": rng.standard_normal((E, C), f32) * 0.05,
        "br": np.zeros(E, f32),
        "w_e": rng.standard_normal((E, C * C * KH * KW), f32) * 0.05,
        "bn2_g": np.ones(C, f32), "bn2_b": np.zeros(C, f32),
        "bn2_m": rng.standard_normal(C, f32) * 0.05,
        "bn2_v": np.abs(rng.standard_normal(C, f32) * 0.05) + 1.0,
    }
    o = kernel(**ins)
    print(o.shape, o.dtype)



# revision 7
# speedup vs baseline: 1.2947x; 1.0245x over previous
"""Trainium2 Bass kernel for Bottleneck+DynamicConv — 1D Winograd F(2,3).

Each 3x3 conv splits into 3 direct row-taps x a width-dim Winograd F(2,3):
per 2-output-column group, 4 Winograd products replace 6 MACs, cutting PE
work to 2/3 of direct convolution (6 vs 9 MACs per output pixel).

Images live as even/odd padded-column planes (E[t]=padded col 2t, O[t]=col
2t+1, 33 each), so the V input transform (4 tensor ops per channel tile and
quarter) reads/writes stride-1 fp16 on the Vector engine. The m-plane
combines (y0=m0+m1+m2, y1=m1-m2-m3) run on GpSimd straight out of PSUM,
SiLU+bias epilogues on the Scalar engine (with accum_out providing the
routing pool for free). Weights carry the Winograd G transform (and BN
scales) folded in: conv1 on the host, the expert-mixed conv2 kernel via 4
vector ops on device. Compute dtype fp16, accumulation fp32.
"""

from contextlib import ExitStack

import numpy as np

import concourse.bacc as bacc
import concourse.bass as bass
import concourse.mybir as mybir
from concourse import tile
from concourse.bass_utils import run_bass_kernel_spmd

B, C, H, W, E = 16, 256, 64, 64, 4
KH = KW = 3
EPS = 1e-5
NCORES = 8
S = B // NCORES           # samples per core = 2
CT = C // 128             # channel tiles = 2
PR = H + 2                # padded rows = 66
PC = W // 2 + 1           # parity-plane cols = 33
PPF = PR * PC             # per-parity plane = 2178
EOF_ = 2 * PPF            # both parities = 4356
G32 = W // 2              # column groups = 32
NQ = 4                    # row quarters
QR = H // NQ              # rows per quarter = 16
NN = QR * G32             # matmul free dim = 512
HWF = H * W
NBLK = CT * 9 * CT        # mixed-kernel blocks (o, t=dy*3+kx, ci)
BLKF = NBLK * 128         # 4608
W1B = CT * 4 * 3 * CT     # conv1 winograd blocks (o, r, dy, ci) = 48
W1F = W1B * 128           # 6144
KWB = CT * 2 * 3 * CT     # device-transformed blocks (o, rr, dy, ci) = 24
KWF = KWB * 128           # 3072
F16 = mybir.dt.float16
F32 = mybir.dt.float32
NPF16 = np.float16

TRACE = False
LAST_EXEC_NS = None
ACT_FUNC = mybir.ActivationFunctionType.Silu

_prog_cache = {}


def _build_program():
    nc = bacc.Bacc(
        "TRN2", target_bir_lowering=False, debug=False,
        enable_asserts=False, num_devices=NCORES)

    xeo_d = nc.dram_tensor("xeo", [S, CT, 128, EOF_], F16, kind="ExternalInput")
    w1t_d = nc.dram_tensor("w1t", [128, W1F], F16, kind="ExternalInput")
    bank_d = nc.dram_tensor("bank", [128, E * W1F], F16, kind="ExternalInput")
    wr_d = nc.dram_tensor("wrt", [128, CT * E], F32, kind="ExternalInput")
    br_d = nc.dram_tensor("brb", [128, E], F32, kind="ExternalInput")
    b1_d = nc.dram_tensor("b1sb", [128, CT], F32, kind="ExternalInput")
    b2_d = nc.dram_tensor("b2sb", [128, CT], F32, kind="ExternalInput")
    id_d = nc.dram_tensor("ident", [128, 128], F16, kind="ExternalInput")
    out_d = nc.dram_tensor("out", [S, CT, 128, HWF], F32, kind="ExternalOutput")

    with tile.TileContext(nc) as tc, ExitStack() as ctx:
        const = ctx.enter_context(tc.tile_pool(name="const", bufs=1))
        eo_pool = ctx.enter_context(tc.tile_pool(name="eo", bufs=2))
        yeo_pool = ctx.enter_context(tc.tile_pool(name="yeo", bufs=2))
        vq_pool = ctx.enter_context(tc.tile_pool(name="vq", bufs=4))
        kern_pool = ctx.enter_context(tc.tile_pool(name="kern", bufs=2))
        cmb_pool = ctx.enter_context(tc.tile_pool(name="cmb", bufs=4))
        outp_pool = ctx.enter_context(tc.tile_pool(name="outp", bufs=4))
        small = ctx.enter_context(tc.tile_pool(name="small", bufs=8))
        ps_pool = ctx.enter_context(tc.tile_pool(name="ps", bufs=7, space="PSUM"))
        psr_pool = ctx.enter_context(tc.tile_pool(name="psr", bufs=1, space="PSUM"))

        w1t_t = const.tile([128, W1F], F16)
        nc.scalar.dma_start(w1t_t[:, 0:W1F // 2], w1t_d.ap()[:, 0:W1F // 2])
        b1_t = const.tile([128, CT], F32)
        nc.sync.dma_start(b1_t[:], b1_d.ap())
        wr_t = const.tile([128, CT * E], F32)
        br_t = const.tile([128, E], F32)
        b2_t = const.tile([128, CT], F32)
        ones_t = const.tile([128, 128], F32)
        nc.vector.memset(ones_t[:], 1.0)
        ident_t = const.tile([128, 128], F16)
        bank_t = const.tile([128, E * W1F], F16)

        # dense dummy matmuls: flip the HAM clock-gate to 2.4 GHz while the
        # startup DMAs stream
        warm_t = const.tile([128, 512], F16)
        nc.vector.memset(warm_t[:], 0.0)
        for wi in range(26):
            wps = ps_pool.tile([128, NN], F32, tag="ps")
            nc.tensor.matmul(wps[:], warm_t[:, 0:128], warm_t[:],
                             start=True, stop=True)

        # row-quarter DMA pieces in consumption order (V for quarter q reads
        # rows 16q..16q+17)
        RQ = [0, 18, 34, 50, PR]
        eos, yeos, kerns = [], [], []
        for s in range(S):
            eot = eo_pool.tile([128, CT * EOF_], F16, tag="eo")
            eos.append(eot)
            pieces = (list(zip(RQ[:-1], RQ[1:])) if s == 0 else [(0, PR)])
            for qi, (lo, hi) in enumerate(pieces):
                for ci in range(CT):
                    for par in range(2):
                        eng = nc.sync if (par == 0 or (s == 0 and qi == 0)) \
                            else nc.scalar
                        base = ci * EOF_ + par * PPF
                        eng.dma_start(
                            eot[:, base + lo * PC:base + hi * PC],
                            xeo_d.ap()[s, ci][:, par * PPF + lo * PC:
                                              par * PPF + hi * PC])
                if s == 0 and qi == 1:
                    nc.scalar.dma_start(
                        w1t_t[:, W1F // 2:], w1t_d.ap()[:, W1F // 2:])
            if s == 0:
                nc.sync.dma_start(wr_t[:], wr_d.ap())
                nc.sync.dma_start(br_t[:], br_d.ap())
                nc.sync.dma_start(b2_t[:], b2_d.ap())
                nc.sync.dma_start(ident_t[:], id_d.ap())

            yeot = yeo_pool.tile([128, CT * EOF_], F16, tag="yeo")
            yeos.append(yeot)
            # pad ring of y: rows 0/65 and the outer parity columns
            for ci in range(CT):
                yv = yeot[:, ci * EOF_:(ci + 1) * EOF_].rearrange(
                    "p (e h w) -> p e h w", e=2, h=PR)
                nc.vector.memset(yv[:, :, 0:1, :], 0.0)
                nc.vector.memset(yv[:, :, PR - 1:PR, :], 0.0)
                nc.vector.memset(yv[:, 0, :, 0:1], 0.0)
                nc.vector.memset(yv[:, 1, :, PC - 1:PC], 0.0)

        def eo_views(t):
            return [t[:, ci * EOF_:(ci + 1) * EOF_].rearrange(
                "p (e h w) -> p e h w", e=2, h=PR) for ci in range(CT)]

        def conv_wg(src_views, wblk, epi):
            for q in range(NQ):
                r0 = 16 * q
                vqs = []
                for ci in range(CT):
                    vq = vq_pool.tile([128, 4 * 18 * G32], F16, tag="vq")
                    vv = vq[:].rearrange("p (r h g) -> p r h g", r=4, h=18)
                    Ev = src_views[ci][:, 0, r0:r0 + 18, :]
                    Ov = src_views[ci][:, 1, r0:r0 + 18, :]
                    nc.vector.tensor_sub(vv[:, 0], Ev[:, :, 0:G32],
                                         Ev[:, :, 1:G32 + 1])
                    nc.vector.tensor_add(vv[:, 1], Ov[:, :, 0:G32],
                                         Ev[:, :, 1:G32 + 1])
                    nc.vector.tensor_sub(vv[:, 2], Ev[:, :, 1:G32 + 1],
                                         Ov[:, :, 0:G32])
                    nc.vector.tensor_sub(vv[:, 3], Ov[:, :, 0:G32],
                                         Ov[:, :, 1:G32 + 1])
                    vqs.append(vv)
                for o in range(CT):
                    pss = []
                    for r in range(4):
                        ps = ps_pool.tile([128, NN], F32, tag="ps")
                        idx = 0
                        for dy in range(3):
                            for ci in range(CT):
                                nc.tensor.matmul(
                                    ps[:], wblk(o, r, dy, ci),
                                    vqs[ci][:, r, dy:dy + QR, :],
                                    start=(idx == 0), stop=(idx == 5))
                                idx += 1
                        pss.append(ps)
                    cO = cmb_pool.tile([128, NN], F32, tag="cmb")
                    cE = cmb_pool.tile([128, NN], F32, tag="cmb")
                    # only one PSUM operand is legal per instruction: seed
                    # each combine with a scalar-engine copy, then accumulate
                    # the remaining m-planes on DVE one bank at a time
                    nc.scalar.activation(
                        cO[:], pss[0][:], mybir.ActivationFunctionType.Copy)
                    nc.vector.tensor_add(cO[:], cO[:], pss[1][:])
                    nc.vector.tensor_add(cO[:], cO[:], pss[2][:])
                    nc.scalar.activation(
                        cE[:], pss[1][:], mybir.ActivationFunctionType.Copy)
                    nc.vector.tensor_sub(cE[:], cE[:], pss[2][:])
                    nc.vector.tensor_sub(cE[:], cE[:], pss[3][:])
                    epi(o, q, cO, cE)

        # ---------------- conv1 for both samples ----------------
        pbcs = [[], []]
        for s in range(S):
            xv = eo_views(eos[s])
            yv = eo_views(yeos[s])
            pps = [small.tile([128, 8], F32, tag="pp", name=f"pp{s}_{o}")
                   for o in range(CT)]

            def epi1(o, q, cO, cE, yv=yv, pps=pps):
                rr = slice(1 + 16 * q, 17 + 16 * q)
                nc.scalar.activation(
                    yv[o][:, 1, rr, 0:G32],
                    cO[:].rearrange("p (a b) -> p a b", a=QR),
                    ACT_FUNC, bias=b1_t[:, o:o + 1],
                    accum_out=pps[o][:, 2 * q:2 * q + 1])
                nc.scalar.activation(
                    yv[o][:, 0, rr, 1:G32 + 1],
                    cE[:].rearrange("p (a b) -> p a b", a=QR),
                    ACT_FUNC, bias=b1_t[:, o:o + 1],
                    accum_out=pps[o][:, 2 * q + 1:2 * q + 2])

            conv_wg(xv, lambda o, r, dy, ci: w1t_t[
                :, (((o * 4 + r) * 3 + dy) * 2 + ci) * 128:
                (((o * 4 + r) * 3 + dy) * 2 + ci + 1) * 128], epi1)

            if s == 0:
                nc.sync.dma_start(bank_t[:], bank_d.ap())

            # routing pool reduce + broadcast (DVE) — emitted here so it's
            # long done when the PE reaches the psr matmuls below
            for ci in range(CT):
                pooled = small.tile([128, 1], F32, tag="pooled")
                nc.vector.tensor_reduce(
                    pooled[:], pps[ci][:],
                    axis=mybir.AxisListType.X, op=mybir.AluOpType.add)
                pbc = small.tile([128, 128], F32, tag="pbc",
                                 name=f"pbc_s{s}_c{ci}")
                nc.vector.tensor_scalar_mul(pbc[:], ones_t[:], pooled[:, 0:1])
                pbcs[s].append(pbc)

        # routing matmul + sigmoid + PE expert mix for both samples, after
        # all conv1 matmuls so the PE never idles on the routing chain:
        # kt_chunk = sum_e diag(r_e).T @ bank_e_chunk accumulated in PSUM,
        # then one scalar Copy evacuates each chunk to fp16
        for s in range(S):
            psr = psr_pool.tile([128, E], F32, tag="psr")
            for ci in range(CT):
                nc.tensor.matmul(
                    psr[:], pbcs[s][ci][:], wr_t[:, ci * E:(ci + 1) * E],
                    start=(ci == 0), stop=(ci == CT - 1))
            logits = small.tile([128, E], F32, tag="logits")
            nc.vector.tensor_add(logits[:], psr[:], br_t[:])
            r_t = small.tile([128, E], F32, tag="r")
            nc.scalar.activation(
                r_t[:], logits[:], mybir.ActivationFunctionType.Sigmoid)
            diags = []
            for e in range(E):
                dg = small.tile([128, 128], F16, tag=f"diag{e}",
                                name=f"diag_s{s}_e{e}")
                nc.vector.tensor_scalar_mul(dg[:], ident_t[:], r_t[:, e:e + 1])
                diags.append(dg)
            kt = kern_pool.tile([128, W1F], F16, tag="kern")
            kerns.append(kt)
            NMC = W1F // NN      # 12 mix chunks of 512 cols
            for mc in range(NMC):
                mps = ps_pool.tile([128, NN], F32, tag="ps")
                for e in range(E):
                    nc.tensor.matmul(
                        mps[:], diags[e][:],
                        bank_t[:, e * W1F + mc * NN:e * W1F + (mc + 1) * NN],
                        start=(e == 0), stop=(e == E - 1))
                nc.scalar.activation(
                    kt[:, mc * NN:(mc + 1) * NN], mps[:],
                    mybir.ActivationFunctionType.Copy)

        # ---------------- conv2 for both samples ----------------
        for s in range(S):
            yv = eo_views(yeos[s])
            xv = eo_views(eos[s])
            kt = kerns[s]

            def kblk(o, r, dy, ci, kt=kt):
                i = ((o * 4 + r) * 3 + dy) * 2 + ci
                return kt[:, i * 128:(i + 1) * 128]

            def epi2(o, q, cO, cE, xv=xv, s=s):
                rr = slice(1 + 16 * q, 17 + 16 * q)
                for par, cc in ((1, cO), (0, cE)):
                    op = outp_pool.tile([128, NN], F32, tag="outp")
                    nc.scalar.activation(
                        op[:], cc[:], ACT_FUNC, bias=b2_t[:, o:o + 1])
                    ov = op[:].rearrange("p (a b) -> p a b", a=QR)
                    if par == 1:
                        nc.gpsimd.tensor_add(ov[:], ov[:],
                                             xv[o][:, 1, rr, 0:G32])
                        off = q * NN
                    else:
                        nc.gpsimd.tensor_add(ov[:], ov[:],
                                             xv[o][:, 0, rr, 1:G32 + 1])
                        off = HWF // 2 + q * NN
                    nc.sync.dma_start(out_d.ap()[s, o][:, off:off + NN], op[:])

            conv_wg(yv, kblk, epi2)

    nc.compile()
    return nc


def _get_program():
    if "nc" not in _prog_cache:
        _prog_cache["nc"] = _build_program()
    return _prog_cache["nc"]


def kernel(x, w1, bn1_g, bn1_b, bn1_m, bn1_v, wr, br, w_e,
           bn2_g, bn2_b, bn2_m, bn2_v):
    global LAST_EXEC_NS
    f32 = np.float32
    x = np.ascontiguousarray(np.asarray(x, f32))
    w1 = np.asarray(w1, f32)
    wr = np.asarray(wr, f32)
    br = np.asarray(br, f32)
    w_e = np.asarray(w_e, f32)

    s1 = np.asarray(bn1_g, f32) / np.sqrt(np.asarray(bn1_v, f32) + EPS)
    b1 = np.asarray(bn1_b, f32) - np.asarray(bn1_m, f32) * s1
    s2 = np.asarray(bn2_g, f32) / np.sqrt(np.asarray(bn2_v, f32) + EPS)
    b2 = np.asarray(bn2_b, f32) - np.asarray(bn2_m, f32) * s2

    # conv1: fold BN1 scale, apply Winograd G along kx, block layout
    # [cin128, (o, r, dy, ci, cout128)]
    w1f = w1 * s1[:, None, None, None]
    g = np.array([[1, 0, 0], [.5, .5, .5], [.5, -.5, .5], [0, 0, 1]], f32)
    w1t = np.einsum('rk,oidk->oidr', g, w1f)          # [cout, cin, dy, r]
    w1sb = np.ascontiguousarray(
        w1t.reshape(CT, 128, CT, 128, 3, 4)
        .transpose(3, 0, 5, 4, 2, 1).reshape(128, W1F)).astype(NPF16)

    # expert bank: BN2 scale and the Winograd G transform folded per expert,
    # same (o, r, dy, ci) block layout as conv1
    wef = w_e.reshape(E, C, C, KH, KW) * s2[None, :, None, None, None]
    weft = np.einsum('rk,eoidk->eoidr', g, wef)      # [E, cout, cin, dy, r]
    bank = np.ascontiguousarray(
        weft.reshape(E, CT, 128, CT, 128, 3, 4)
        .transpose(4, 0, 1, 6, 5, 3, 2).reshape(128, E * W1F)).astype(NPF16)

    wrt = np.ascontiguousarray(
        (wr / HWF).reshape(E, CT, 128).transpose(2, 1, 0).reshape(128, CT * E))
    brb = np.ascontiguousarray(np.broadcast_to(br, (128, E)))
    b1sb = np.ascontiguousarray(b1.reshape(CT, 128).T)
    b2sb = np.ascontiguousarray(b2.reshape(CT, 128).T)

    # x as even/odd padded-column planes [B, CT, 128, 2(par E,O), 66, 33]
    pad = np.zeros((B, CT, 128, PR, PR), f32)
    pad[:, :, :, 1:H + 1, 1:W + 1] = x.reshape(B, CT, 128, H, W)
    xeo = np.empty((B, CT, 128, 2, PR, PC), f32)
    xeo[:, :, :, 0] = pad[:, :, :, :, 0::2]
    xeo[:, :, :, 1] = pad[:, :, :, :, 1::2]
    xeo = np.ascontiguousarray(
        xeo.reshape(B, CT, 128, EOF_).astype(NPF16))

    nc = _get_program()
    in_maps = []
    for c in range(NCORES):
        sl = slice(S * c, S * (c + 1))
        in_maps.append({
            "xeo": np.ascontiguousarray(xeo[sl]),
            "w1t": w1sb, "bank": bank, "wrt": wrt, "brb": brb,
            "b1sb": b1sb, "b2sb": b2sb,
            "ident": np.eye(128, dtype=NPF16),
        })

    res = run_bass_kernel_spmd(
        nc, in_maps, core_ids=list(range(NCORES)), trace=TRACE)
    LAST_EXEC_NS = res.exec_time_ns

    out = np.empty((B, C, H, W), f32)
    for c in range(NCORES):
        dev = res.results[c]["out"].reshape(S, CT, 128, 2, H, G32)
        img = np.empty((S, CT, 128, H, W), f32)
        img[..., 0::2] = dev[:, :, :, 0]
        img[..., 1::2] = dev[:, :, :, 1]
        out[S * c:S * (c + 1)] = img.reshape(S, C, H, W)
    return out
